# revision 36
# baseline (speedup 1.0000x reference)
"""Trainium2 Bass kernel for AttentionWithRelPos.

Strategy: data-parallel over batch B=16 across 8 NeuronCores (2 batches/core).
Per core, attention is computed in "S^T" orientation (keys on partitions,
queries on the free dim) so the P@V matmul needs no transposes:

  - qkv projection emits Q^T/K^T in [d, token] layout and V in [token, d]
    layout (both from one x^T load).
  - Keys processed in 5 chunks over tokens: [CLS+120 patches, 120, 120, 120,
    96].
  - Mask bias and rel-pos bias are pre-merged on the host into ONE per
    (batch, head) fp8 table in key-major layout; it is accumulated into the
    QK PSUM with a single identity-matmul per chunk/bank (halves the PE
    stream vs separate bias+mask matmuls, and fp8 halves the table DMA).
  - Softmax division: 1/den = exp(-ln den) on the scalar engine (the Exp
    activation is routed to the table set that also holds Ln, so no table
    reloads); each head's reciprocal chain is deferred by one head so it
    never sits in the ACT FIFO ahead of the next head's chunk exps.
  - PSUM accumulators ping-pong between two pools so back-to-back matmul
    groups never serialize; batch 1's qkv phase is hoisted before batch 0's
    output projection to keep the PE fed while division chains drain.
  - Softmax without max-subtraction (logits are provably tiny for this
    problem's distributions; masked entries get a large negative bias ->
    exp underflows to 0).
  - exp() runs on the scalar engine straight out of PSUM; denominator comes
    free from an appended ones-column in V'; division happens once on the
    65x577 head output.
  - Matmuls run as float32r / bf16 (1 PE cycle per output column; the query
    axis is padded 577 -> 580 = 2 banks x 290).
"""

import sys

if '/opt/trn_rl_repo' not in sys.path:
    sys.path.insert(0, '/opt/trn_rl_repo')

import numpy as np
import ml_dtypes

import concourse.bass as bass
import concourse.mybir as mybir
from concourse import bacc
from concourse.tile import TileContext
from concourse.masks import make_identity
from concourse import bass_utils
import concourse.hw_specs as _hw_specs
import concourse.bass_interp as _bass_interp

# Route Exp to the activation table that also holds Ln ("natural_log_exp_
# and_others"), so the softmax exp and the exp(-ln x) reciprocal share one
# table and the scalar engine never reloads tables mid-kernel. Table-set
# indices are preserved (sets keep their positions; Exp is just removed
# from the earlier sets so first-match lands on the combined one).
_ORIG_GAT = _hw_specs.get_activation_tables


def _gat_one_exp_table(arch):
    tabs = _ORIG_GAT(arch)
    exp = mybir.ActivationFunctionType.Exp
    out = {}
    for name, s in tabs.items():
        if exp in s and name != 'natural_log_exp_and_others':
            s = set(s) - {exp}
        out[name] = s
    return out


_hw_specs.get_activation_tables = _gat_one_exp_table
bacc.get_activation_tables = _gat_one_exp_table
_bass_interp.get_activation_tables = _gat_one_exp_table

B, N, C, H = 16, 577, 384, 6
NQ = 580                    # padded query axis (2 banks x 290)
HEAD_DIM = C // H           # 64
SCALE = HEAD_DIM ** -0.5
NB = 2                      # batches per core
NCORES = 8
NUM_CLS = 1
F32 = mybir.dt.float32
F32R = mybir.dt.float32r
BF16 = mybir.dt.bfloat16
FP8 = mybir.dt.float8e4
FP8NP = ml_dtypes.float8_e4m3

# key chunks in token space: (token0, rows). Chunk 0 includes CLS.
# 128-aligned so QK/acc weight loads are full 128-col (Fast Weight Load).
CHUNKS = [(0, 128), (128, 128), (256, 128), (384, 128), (512, 65)]
QW = 290                    # query columns per psum bank
QWP = 304                   # fp8 table bank pitch (16B-aligned for DoubleRow)
DR = mybir.MatmulPerfMode.DoubleRow
QK_BF16 = True              # Q^T/K^T tiles + QK matmuls in bf16
ACC_MODE = 'bf16'           # merged-table accumulate path ('bf16' branch
                            # now stores the table in plain fp8; 'fp8dr'
                            # DoubleRow is broken on this HW/runtime)
REC_FAST = True             # use reciprocal_approx_fast


def _mm(nc, out, lhsT, rhs, **kw):
    nc.tensor.matmul(out, lhsT.bitcast(F32R), rhs.bitcast(F32R),
                     skip_group_check=True, **kw)


def _mmb(nc, out, lhsT, rhs, **kw):
    nc.tensor.matmul(out, lhsT, rhs, skip_group_check=True, **kw)


def _emit_div(nc, x2t, h, ti, po, ocp):
    """ln->exp reciprocal of the denominator row, broadcast, normalize."""
    spool = _emit_div.spool
    QW_ = 290
    rec = spool.tile([1, 2, QW_], mybir.dt.float32, tag="rec", bufs=2,
                     name=f"rec_{h}")
    lnt = spool.tile([1, 2, QW_], mybir.dt.float32, tag="lnt", bufs=2,
                     name=f"lnt_{h}")
    nc.scalar.activation(lnt[:, :, :], ocp[64:65, :, :],
                         mybir.ActivationFunctionType.Ln)
    nc.scalar.activation(rec[:, :, :], lnt[:, :, :],
                         mybir.ActivationFunctionType.Exp, scale=-1.0)
    recb = spool.tile([64, 2, QW_], mybir.dt.float32, tag="recb", bufs=3,
                      name=f"recb_{h}")
    nc.gpsimd.partition_broadcast(recb[:, :, :], rec[:, :, :])
    nc.vector.tensor_tensor(x2t[ti][po:po + 64, :, :], ocp[:64, :, :],
                            recb[:, :, :], mybir.AluOpType.mult)


def build_program(patch_attn: bool):
    nc = bacc.Bacc("TRN2", target_bir_lowering=False, debug=False,
                   enable_asserts=False)

    xT = nc.dram_tensor("xT", [NB, C, NQ], F32R, kind="ExternalInput")
    # merged mask+rel-pos bias table, per (batch, head), key-major
    if ACC_MODE == 'fp8dr':
        # [j, chunk, bank, DR-subrow, col]; strides 16B-aligned for DoubleRow
        mbt_d = nc.dram_tensor("mbt", [NB, H, 64, 5, 2, 2, QWP], FP8,
                               kind="ExternalInput")
        idr_d = nc.dram_tensor("idr", [64, 2, 128], FP8, kind="ExternalInput")
    else:
        mbt_d = nc.dram_tensor("mbt", [NB, H, 128, 5, NQ], FP8,
                               kind="ExternalInput")
        id8_d = nc.dram_tensor("id8", [128, 128], FP8, kind="ExternalInput")
    qkv_wT = nc.dram_tensor("qkv_wT", [C, 3 * C], F32R, kind="ExternalInput")
    proj_wT = nc.dram_tensor("proj_wT", [C, C], F32R, kind="ExternalInput")
    qkvb_qk = nc.dram_tensor("qkvb_qk", [2 * C, 1], F32, kind="ExternalInput")
    qkvbv_bc = nc.dram_tensor("qkvbv_bc", [128, C], F32, kind="ExternalInput")
    projb_bc = nc.dram_tensor("projb_bc", [128, C], F32, kind="ExternalInput")
    vpad = nc.dram_tensor("vpad", [128, H, 1], F32R, kind="ExternalInput")
    out_d = nc.dram_tensor("out", [NB, N, C], F32, kind="ExternalOutput")

    with TileContext(nc) as tc:
        with (
            tc.tile_pool(name="const", bufs=1) as cpool,
            tc.tile_pool(name="batch", bufs=2) as bpool,
            tc.tile_pool(name="ptile", bufs=3) as ppool,
            tc.tile_pool(name="small", bufs=1) as spool,
            tc.tile_pool(name="s_psum", bufs=2, space="PSUM") as s_pool,
            tc.tile_pool(name="o_psum", bufs=1, space="PSUM") as o_pool,
            tc.tile_pool(name="mm_psum", bufs=1, space="PSUM") as m_pool,
        ):
            _emit_div.spool = spool
            # ---------------- constants ----------------
            if ACC_MODE == 'fp8dr':
                # fp8 DoubleRow identity: idr[k,r,m]=1 iff m==2k+r
                idr = cpool.tile([64, 2, 128], FP8, tag="idr")
                nc.sync.dma_start(idr[:], idr_d[:])
            else:
                ident = cpool.tile([128, 128], BF16, tag="ident")
                make_identity(nc, ident[:])

            wqkv = []
            for ci in range(3):
                t = cpool.tile([128, 3 * C], F32R, tag=f"wqkv{ci}")
                nc.sync.dma_start(t[:], qkv_wT[128 * ci:128 * (ci + 1), :])
                wqkv.append(t)
            wproj = []
            for ci in range(3):
                t = cpool.tile([128, C], F32R, tag=f"wproj{ci}")
                nc.sync.dma_start(t[:], proj_wT[128 * ci:128 * (ci + 1), :])
                wproj.append(t)
            bqk = []
            for oi in range(6):
                t = cpool.tile([128, 1], F32, tag=f"bqk{oi}")
                nc.sync.dma_start(t[:], qkvb_qk[128 * oi:128 * (oi + 1), :])
                bqk.append(t)
            bv = cpool.tile([128, C], F32, tag="bv")
            nc.sync.dma_start(bv[:], qkvbv_bc[:, :])
            bpj = cpool.tile([128, C], F32, tag="bpj")
            nc.sync.dma_start(bpj[:], projb_bc[:, :])
            vpd = cpool.tile([128, H, 1], F32R, tag="vpd")
            nc.sync.dma_start(vpd[:], vpad[:])

            # psum ping-pong between the two 2-bank pools so back-to-back
            # matmul groups never serialize on a single accumulator
            def mm_ps(i):
                if i % 2 == 0:
                    return m_pool.tile([128, 2, 512], F32, tag="mm",
                                       name="mmps")
                return s_pool.tile([128, 2, 512], F32, tag="sp", name="spps")

            qkts = {}
            vtss = {}
            x2ts = {}
            xtss = {}

            mmctr = [0]

            def emit_qkt(b, oi):
                xts = xtss[b]
                ps = mm_ps(mmctr[0])
                mmctr[0] += 1
                for bk in range(2):
                    for ci in range(3):
                        _mm(nc, ps[:, bk, :QW],
                            wqkv[ci][:, 128 * oi:128 * (oi + 1)],
                            xts[ci][:, QW * bk:QW * (bk + 1)],
                            start=(ci == 0), stop=(ci == 2))
                t = bpool.tile([128, 2, QW], BF16 if QK_BF16 else F32R,
                               tag=f"qkt{oi}", name=f"qkt{oi}_{b}")
                nc.vector.tensor_scalar_add(t[:, :, :], ps[:, :, :QW],
                                            bqk[oi][:])
                qkts[b][oi] = t

            def emit_qkv(b):
                # ---------------- load x^T ----------------
                xts = []
                for ci in range(3):
                    t = bpool.tile([128, NQ], F32R, tag=f"xt{ci}")
                    nc.sync.dma_start(t[:], xT[b, 128 * ci:128 * (ci + 1), :])
                    xts.append(t)
                xtss[b] = xts

                qkts[b] = {}

                x2ts[b] = [bpool.tile([128, 2, QW], F32R, tag=f"x2t{ci}",
                                      name=f"x2t{ci}_{b}") for ci in range(3)]

            def emit_vts(b):
                xts = xtss[b]
                # ---------------- qkv projection: V (token-major) ----------
                vts = []
                for c, (t0, rows) in enumerate(CHUNKS):
                    ps = mm_ps(mmctr[0])
                    mmctr[0] += 1
                    for ci in range(3):
                        _mm(nc, ps[:rows, 0, :C], xts[ci][:, t0:t0 + rows],
                            wqkv[ci][:, 2 * C:3 * C],
                            start=(ci == 0), stop=(ci == 2))
                    t = bpool.tile([128, H, HEAD_DIM + 1], F32R, tag=f"vt{c}")
                    nc.vector.tensor_tensor(
                        t[:rows, :, :HEAD_DIM],
                        ps[:rows, 0, :C].rearrange("p (h d) -> p h d", h=H),
                        bv[:rows, :].rearrange("p (h d) -> p h d", h=H),
                        mybir.AluOpType.add)
                    # ones column for the denominator row
                    nc.sync.dma_start(t[:rows, :, HEAD_DIM:HEAD_DIM + 1],
                                      vpd[:rows, :, :])
                    vts.append(t)
                vtss[b] = vts

            pend = []

            def emit_head(b, h):
                qkt, vts, x2t = qkts[b], vtss[b], x2ts[b]
                if True:
                    ti, po = h // 2, 64 * (h % 2)
                    qT = qkt[ti][po:po + 64, :, :].rearrange("p a b -> p (a b)")
                    kT = qkt[3 + ti][po:po + 64, :, :].rearrange("p a b -> p (a b)")
                    # merged mask+bias table for this (batch, head)
                    if ACC_MODE == 'fp8dr':
                        mt = bpool.tile([64, 5, 2, 2, QWP], FP8, tag="mbt",
                                        bufs=3)
                    else:
                        mt = bpool.tile([128, 5, NQ], FP8, tag="mbt", bufs=3)
                    nc.gpsimd.dma_start(mt[:], mbt_d[b, h])
                    ov = o_pool.tile([65, 2, 512], F32, tag="ov")
                    for c, (t0, rows) in enumerate(CHUNKS):
                        kdr = (rows + 1) // 2
                        sp = s_pool.tile([128, 2, 512], F32, tag="sp")
                        mmqk = _mmb if QK_BF16 else _mm
                        for bk in range(2):
                            mmqk(nc, sp[:rows, bk, :QW], kT[:, t0:t0 + rows],
                                 qT[:, QW * bk:QW * (bk + 1)],
                                 start=True, stop=False)
                        for bk in range(2):
                            if ACC_MODE == 'fp8dr':
                                _mmb(nc, sp[:rows, bk, :QW],
                                     idr[:kdr, :, :rows],
                                     mt[:kdr, c, bk, :, :QW],
                                     perf_mode=DR, start=False, stop=True)
                            else:
                                _mmb(nc, sp[:rows, bk, :QW],
                                     ident[:rows, :rows],
                                     mt[:rows, c, QW * bk:QW * (bk + 1)],
                                     start=False, stop=True)
                        # exp
                        pt = ppool.tile([128, 2, QW], F32R, tag="pt")
                        nc.scalar.activation(
                            pt[:rows, :, :], sp[:rows, :, :QW],
                            mybir.ActivationFunctionType.Exp)
                        # P^T @ V'
                        for bk in range(2):
                            _mm(nc, ov[:, bk, :QW], vts[c][:rows, h, :],
                                pt[:rows, bk, :],
                                start=(c == 0), stop=(c == 4))
                    # copy out of PSUM to free the accumulator early
                    ocp = spool.tile([65, 2, QW], F32, tag="ocp", bufs=3)
                    nc.vector.tensor_copy(ocp[:, :, :], ov[:, :, :QW])
                    # defer this head's reciprocal chain by one head so its
                    # scalar-engine ops never sit in the ACT FIFO ahead of
                    # the next head's chunk exps
                    pend.append((x2t, h, ti, po, ocp))
                    if len(pend) > 1:
                        _emit_div(nc, *pend.pop(0))

            def emit_proj(b):
                x2t = x2ts[b]
                tsl = [(0, 128), (128, 128), (256, 128), (384, 128), (512, 65)]
                for i, (t0, tn) in enumerate(tsl):
                    ps = mm_ps(mmctr[0])
                    mmctr[0] += 1
                    for ci in range(3):
                        _mm(nc, ps[:tn, 0, :C],
                            x2t[ci][:, :, :].rearrange("p a b -> p (a b)")[:, t0:t0 + tn],
                            wproj[ci][:, :], start=(ci == 0), stop=(ci == 2))
                    yt = spool.tile([128, C], F32, tag="yt", bufs=3)
                    nc.vector.tensor_tensor(yt[:tn, :], ps[:tn, 0, :C],
                                            bpj[:tn, :], mybir.AluOpType.add)
                    nc.sync.dma_start(out_d[b, t0:t0 + tn, :], yt[:tn, :])

            # schedule: emit each batch's qkt weight-groups just-in-time
            # before the head pair that consumes them, so attention ramps
            # while the remaining projections stream; hoist batch 1's qkv
            # before batch 0's proj to cover the division-chain tail
            emit_qkv(0)
            emit_qkt(0, 0)
            emit_qkt(0, 3)
            emit_vts(0)
            emit_head(0, 0)
            emit_qkt(0, 1)
            emit_qkt(0, 4)
            emit_head(0, 1)
            emit_head(0, 2)
            emit_qkt(0, 2)
            emit_qkt(0, 5)
            emit_head(0, 3)
            emit_head(0, 4)
            emit_head(0, 5)
            emit_qkv(1)
            for oi in (0, 3, 1, 4, 2, 5):
                emit_qkt(1, oi)
            emit_vts(1)
            while pend:
                _emit_div(nc, *pend.pop(0))
            emit_proj(0)
            for h in range(H):
                emit_head(1, h)
            while pend:
                _emit_div(nc, *pend.pop(0))
            emit_proj(1)

    nc.compile()
    return nc


def prep_inputs(x, qkv_w, qkv_b, proj_w, proj_b, rel_pos, rel_pos_index,
                mask, patch_attn):
    x = np.asarray(x, dtype=np.float32)
    qkv_w = np.asarray(qkv_w, dtype=np.float32)
    qkv_b = np.asarray(qkv_b, dtype=np.float32)
    proj_w = np.asarray(proj_w, dtype=np.float32)
    proj_b = np.asarray(proj_b, dtype=np.float32)
    rel_pos = np.asarray(rel_pos, dtype=np.float32)
    mask = np.asarray(mask)

    # x^T padded to 580 query columns (zeros in the pad)
    xT = np.zeros((B, C, NQ), dtype=np.float32)
    xT[:, :, :N] = x.transpose(0, 2, 1)
    W = qkv_w.copy()
    W[:C] *= np.float32(SCALE)
    b2 = qkv_b.copy()
    b2[:C] *= np.float32(SCALE)
    qkv_wT = np.ascontiguousarray(W.T)
    proj_wT = np.ascontiguousarray(proj_w.T)
    qkvb_qk = np.ascontiguousarray(b2[:2 * C].reshape(2 * C, 1))
    qkvbv_bc = np.ascontiguousarray(np.broadcast_to(b2[2 * C:], (128, C)))
    projb_bc = np.ascontiguousarray(np.broadcast_to(proj_b, (128, C)))
    vpad = np.ones((128, H, 1), dtype=np.float32)

    # fp8 DoubleRow identity
    k_ = np.arange(64)
    idr = np.zeros((64, 2, 128), dtype=FP8NP)
    for r in range(2):
        idr[k_, r, np.minimum(2 * k_ + r, 127)] = (2 * k_ + r < 128)

    # merged mask + rel-pos bias table, key-major
    MASKVAL = 240.0
    relb = np.zeros((H, N, NQ), dtype=np.float32)
    if patch_attn:
        relb[:, NUM_CLS:, NUM_CLS:N] = rel_pos[:, rel_pos_index.T]
    mb = (mask.transpose(0, 2, 1).astype(np.float32) - 1.0) * MASKVAL  # [B,k,q]
    if ACC_MODE == 'fp8dr':
        # DR-packed: mbt[b,h,j,c,bk,r,col] = bias[b,h, key=t0_c+2j+r,
        #                                         q=290*bk+col]
        mbt = np.empty((B, H, 5, 64, 2, 2, QWP), dtype=FP8NP)
        pk = np.empty((H, 5, 128, 2, QWP), dtype=np.float32)
        for b in range(B):
            pk[:] = 0.0
            for c, (t0, rows) in enumerate(CHUNKS):
                blk = relb[:, t0:t0 + rows, :].copy()
                blk[:, :, :N] += mb[b, t0:t0 + rows, :][None]
                for bk in range(2):
                    pk[:, c, :rows, bk, :QW] = blk[:, :, QW * bk:QW * (bk + 1)]
            # (h, c, key=2j+r, bk, col) -> (h, c, j, r, bk, col)
            pk8 = pk.reshape(H, 5, 64, 2, 2, QWP).astype(FP8NP)
            mbt[b] = pk8.transpose(0, 1, 2, 4, 3, 5)  # swap r <-> bk
        mbt = np.ascontiguousarray(mbt.transpose(0, 1, 3, 2, 4, 5, 6))
    else:
        # mbt[b, h, k, c, q] = bias[b, h, key = t0_c + k, q]
        mbt = np.empty((B, H, 5, 128, NQ), dtype=FP8NP)
        pk = np.empty((H, 5, 128, NQ), dtype=np.float32)
        for b in range(B):
            pk[:] = 0.0
            for c, (t0, rows) in enumerate(CHUNKS):
                pk[:, c, :rows, :] = relb[:, t0:t0 + rows, :]
                pk[:, c, :rows, :N] += mb[b, t0:t0 + rows, :][None]
            mbt[b] = pk.astype(FP8NP)
        mbt = np.ascontiguousarray(mbt.transpose(0, 1, 3, 2, 4))

    shared = {
        "qkv_wT": qkv_wT, "proj_wT": proj_wT,
        "qkvb_qk": qkvb_qk, "qkvbv_bc": qkvbv_bc, "projb_bc": projb_bc,
        "vpad": vpad, "id8": np.eye(128, dtype=FP8NP),
    }
    in_maps = []
    for i in range(NCORES):
        m = dict(shared)
        m["xT"] = np.ascontiguousarray(xT[NB * i:NB * (i + 1)])
        m["mbt"] = np.ascontiguousarray(mbt[NB * i:NB * (i + 1)])
        in_maps.append(m)
    return in_maps


_NC_CACHE = {}


def _get_nc(patch_attn: bool):
    key = (bool(patch_attn), QK_BF16, ACC_MODE, REC_FAST)
    if key not in _NC_CACHE:
        _NC_CACHE[key] = build_program(bool(patch_attn))
    return _NC_CACHE[key]


def kernel(**inputs):
    patch_attn = bool(np.asarray(inputs["patch_attn"]))
    nc = _get_nc(patch_attn)
    in_maps = prep_inputs(**inputs)
    res = bass_utils.run_bass_kernel_spmd(nc, in_maps,
                                          core_ids=list(range(NCORES)))
    out = np.concatenate([res.results[i]["out"] for i in range(NCORES)], axis=0)
    return np.ascontiguousarray(out.astype(np.float32))


# revision 37
# speedup vs baseline: 1.0566x; 1.0566x over previous
"""Trainium2 Bass kernel for AttentionWithRelPos.

Strategy: data-parallel over batch B=16 across 8 NeuronCores (2 batches/core).
Per core, attention is computed in "S^T" orientation (keys on partitions,
queries on the free dim) so the P@V matmul needs no transposes:

  - qkv projection emits Q^T/K^T in [d, token] layout and V in [token, d]
    layout (both from one x^T load).
  - Keys processed in 5 chunks over tokens: [CLS+120 patches, 120, 120, 120,
    96].
  - Mask bias and rel-pos bias are pre-merged on the host into ONE per
    (batch, head) fp8 table in key-major layout; it is accumulated into the
    QK PSUM with a single identity-matmul per chunk/bank (halves the PE
    stream vs separate bias+mask matmuls, and fp8 halves the table DMA).
  - Softmax division: 1/den = exp(-ln den) on the scalar engine (the Exp
    activation is routed to the table set that also holds Ln, so no table
    reloads); each head's reciprocal chain is deferred by one head so it
    never sits in the ACT FIFO ahead of the next head's chunk exps.
  - PSUM accumulators ping-pong between two pools so back-to-back matmul
    groups never serialize; batch 1's qkv phase is hoisted before batch 0's
    output projection to keep the PE fed while division chains drain.
  - Softmax without max-subtraction (logits are provably tiny for this
    problem's distributions; masked entries get a large negative bias ->
    exp underflows to 0).
  - exp() runs on the scalar engine straight out of PSUM; denominator comes
    free from an appended ones-column in V'; division happens once on the
    65x577 head output.
  - Matmuls run as float32r / bf16 (1 PE cycle per output column; the query
    axis is padded 577 -> 580 = 2 banks x 290).
"""

import sys

if '/opt/trn_rl_repo' not in sys.path:
    sys.path.insert(0, '/opt/trn_rl_repo')

import numpy as np
import ml_dtypes

import concourse.bass as bass
import concourse.mybir as mybir
from concourse import bacc
from concourse.tile import TileContext
from concourse.masks import make_identity
from concourse import bass_utils
import concourse.hw_specs as _hw_specs
import concourse.bass_interp as _bass_interp

# Route Exp to the activation table that also holds Ln ("natural_log_exp_
# and_others"), so the softmax exp and the exp(-ln x) reciprocal share one
# table and the scalar engine never reloads tables mid-kernel. Table-set
# indices are preserved (sets keep their positions; Exp is just removed
# from the earlier sets so first-match lands on the combined one).
_ORIG_GAT = _hw_specs.get_activation_tables


def _gat_one_exp_table(arch):
    tabs = _ORIG_GAT(arch)
    exp = mybir.ActivationFunctionType.Exp
    out = {}
    for name, s in tabs.items():
        if exp in s and name != 'natural_log_exp_and_others':
            s = set(s) - {exp}
        out[name] = s
    return out


_hw_specs.get_activation_tables = _gat_one_exp_table
bacc.get_activation_tables = _gat_one_exp_table
_bass_interp.get_activation_tables = _gat_one_exp_table

B, N, C, H = 16, 577, 384, 6
NQ = 580                    # padded query axis (2 banks x 290)
HEAD_DIM = C // H           # 64
SCALE = HEAD_DIM ** -0.5
NB = 2                      # batches per core
NCORES = 8
NUM_CLS = 1
F32 = mybir.dt.float32
F32R = mybir.dt.float32r
BF16 = mybir.dt.bfloat16
FP8 = mybir.dt.float8e4
FP8NP = ml_dtypes.float8_e4m3

# key chunks in token space: (token0, rows). Chunk 0 includes CLS.
# 128-aligned so QK/acc weight loads are full 128-col (Fast Weight Load).
CHUNKS = [(0, 128), (128, 128), (256, 128), (384, 128), (512, 65)]
QW = 290                    # query columns per psum bank
QWP = 304                   # fp8 table bank pitch (16B-aligned for DoubleRow)
DR = mybir.MatmulPerfMode.DoubleRow
QK_BF16 = True              # Q^T/K^T tiles + QK matmuls in bf16
ACC_MODE = 'bf16'           # merged-table accumulate path ('bf16' branch
                            # now stores the table in plain fp8; 'fp8dr'
                            # DoubleRow is broken on this HW/runtime)
REC_FAST = True             # use reciprocal_approx_fast


def _mm(nc, out, lhsT, rhs, **kw):
    nc.tensor.matmul(out, lhsT.bitcast(F32R), rhs.bitcast(F32R),
                     skip_group_check=True, **kw)


def _mmb(nc, out, lhsT, rhs, **kw):
    nc.tensor.matmul(out, lhsT, rhs, skip_group_check=True, **kw)


def _emit_div(nc, x2t, h, ti, po, ocp):
    """ln->exp reciprocal of the denominator row, broadcast, normalize."""
    spool = _emit_div.spool
    QW_ = 290
    rec = spool.tile([1, 2, QW_], mybir.dt.float32, tag="rec", bufs=2,
                     name=f"rec_{h}")
    lnt = spool.tile([1, 2, QW_], mybir.dt.float32, tag="lnt", bufs=2,
                     name=f"lnt_{h}")
    nc.scalar.activation(lnt[:, :, :], ocp[64:65, :, :],
                         mybir.ActivationFunctionType.Ln)
    nc.scalar.activation(rec[:, :, :], lnt[:, :, :],
                         mybir.ActivationFunctionType.Exp, scale=-1.0)
    recb = spool.tile([64, 2, QW_], mybir.dt.float32, tag="recb", bufs=3,
                      name=f"recb_{h}")
    nc.gpsimd.partition_broadcast(recb[:, :, :], rec[:, :, :])
    nc.vector.tensor_tensor(x2t[ti][po:po + 64, :, :], ocp[:64, :, :],
                            recb[:, :, :], mybir.AluOpType.mult)


def build_program(patch_attn: bool):
    nc = bacc.Bacc("TRN2", target_bir_lowering=False, debug=False,
                   enable_asserts=False)

    xT = nc.dram_tensor("xT", [NB, C, NQ], BF16, kind="ExternalInput")
    # merged mask+rel-pos bias table, per (batch, head), key-major
    if ACC_MODE == 'fp8dr':
        # [j, chunk, bank, DR-subrow, col]; strides 16B-aligned for DoubleRow
        mbt_d = nc.dram_tensor("mbt", [NB, H, 64, 5, 2, 2, QWP], FP8,
                               kind="ExternalInput")
        idr_d = nc.dram_tensor("idr", [64, 2, 128], FP8, kind="ExternalInput")
    else:
        mbt_d = nc.dram_tensor("mbt", [NB, H, 128, 5, NQ], FP8,
                               kind="ExternalInput")
        id8_d = nc.dram_tensor("id8", [128, 128], FP8, kind="ExternalInput")
    qkv_wT = nc.dram_tensor("qkv_wT", [C, 3 * C], BF16, kind="ExternalInput")
    proj_wT = nc.dram_tensor("proj_wT", [C, C], BF16, kind="ExternalInput")
    qkvb_qk = nc.dram_tensor("qkvb_qk", [2 * C, 1], F32, kind="ExternalInput")
    qkvbv_bc = nc.dram_tensor("qkvbv_bc", [128, C], F32, kind="ExternalInput")
    projb_bc = nc.dram_tensor("projb_bc", [128, C], F32, kind="ExternalInput")
    vpad = nc.dram_tensor("vpad", [128, H, 1], F32R, kind="ExternalInput")
    out_d = nc.dram_tensor("out", [NB, N, C], F32, kind="ExternalOutput")

    with TileContext(nc) as tc:
        with (
            tc.tile_pool(name="const", bufs=1) as cpool,
            tc.tile_pool(name="batch", bufs=2) as bpool,
            tc.tile_pool(name="ptile", bufs=3) as ppool,
            tc.tile_pool(name="small", bufs=1) as spool,
            tc.tile_pool(name="s_psum", bufs=2, space="PSUM") as s_pool,
            tc.tile_pool(name="o_psum", bufs=1, space="PSUM") as o_pool,
            tc.tile_pool(name="mm_psum", bufs=1, space="PSUM") as m_pool,
        ):
            _emit_div.spool = spool
            # ---------------- constants ----------------
            if ACC_MODE == 'fp8dr':
                # fp8 DoubleRow identity: idr[k,r,m]=1 iff m==2k+r
                idr = cpool.tile([64, 2, 128], FP8, tag="idr")
                nc.sync.dma_start(idr[:], idr_d[:])
            else:
                ident = cpool.tile([128, 128], BF16, tag="ident")
                make_identity(nc, ident[:])

            wqkv = []
            for ci in range(3):
                t = cpool.tile([128, 3 * C], BF16, tag=f"wqkv{ci}")
                nc.sync.dma_start(t[:], qkv_wT[128 * ci:128 * (ci + 1), :])
                wqkv.append(t)
            wproj = []
            for ci in range(3):
                t = cpool.tile([128, C], BF16, tag=f"wproj{ci}")
                nc.sync.dma_start(t[:], proj_wT[128 * ci:128 * (ci + 1), :])
                wproj.append(t)
            bqk = []
            for oi in range(6):
                t = cpool.tile([128, 1], F32, tag=f"bqk{oi}")
                nc.sync.dma_start(t[:], qkvb_qk[128 * oi:128 * (oi + 1), :])
                bqk.append(t)
            bv = cpool.tile([128, C], F32, tag="bv")
            nc.sync.dma_start(bv[:], qkvbv_bc[:, :])
            bpj = cpool.tile([128, C], F32, tag="bpj")
            nc.sync.dma_start(bpj[:], projb_bc[:, :])
            vpd = cpool.tile([128, H, 1], F32R, tag="vpd")
            nc.sync.dma_start(vpd[:], vpad[:])

            # psum ping-pong between the two 2-bank pools so back-to-back
            # matmul groups never serialize on a single accumulator
            def mm_ps(i):
                if i % 2 == 0:
                    return m_pool.tile([128, 2, 512], F32, tag="mm",
                                       name="mmps")
                return s_pool.tile([128, 2, 512], F32, tag="sp", name="spps")

            qkts = {}
            vtss = {}
            x2ts = {}
            xtss = {}

            mmctr = [0]

            def emit_qkt(b, oi):
                xts = xtss[b]
                ps = mm_ps(mmctr[0])
                mmctr[0] += 1
                for bk in range(2):
                    for ci in range(3):
                        _mmb(nc, ps[:, bk, :QW],
                             wqkv[ci][:, 128 * oi:128 * (oi + 1)],
                             xts[ci][:, QW * bk:QW * (bk + 1)],
                             start=(ci == 0), stop=(ci == 2))
                t = bpool.tile([128, 2, QW], BF16 if QK_BF16 else F32R,
                               tag=f"qkt{oi}", name=f"qkt{oi}_{b}")
                nc.vector.tensor_scalar_add(t[:, :, :], ps[:, :, :QW],
                                            bqk[oi][:])
                qkts[b][oi] = t

            def emit_qkv(b):
                # ---------------- load x^T ----------------
                xts = []
                for ci in range(3):
                    t = bpool.tile([128, NQ], BF16, tag=f"xt{ci}")
                    nc.sync.dma_start(t[:], xT[b, 128 * ci:128 * (ci + 1), :])
                    xts.append(t)
                xtss[b] = xts

                qkts[b] = {}

                x2ts[b] = [bpool.tile([128, 2, QW], BF16, tag=f"x2t{ci}",
                                      name=f"x2t{ci}_{b}") for ci in range(3)]

            def emit_vts(b):
                xts = xtss[b]
                # ---------------- qkv projection: V (token-major) ----------
                vts = []
                for c, (t0, rows) in enumerate(CHUNKS):
                    ps = mm_ps(mmctr[0])
                    mmctr[0] += 1
                    for ci in range(3):
                        _mmb(nc, ps[:rows, 0, :C], xts[ci][:, t0:t0 + rows],
                             wqkv[ci][:, 2 * C:3 * C],
                             start=(ci == 0), stop=(ci == 2))
                    t = bpool.tile([128, H, HEAD_DIM + 1], F32R, tag=f"vt{c}")
                    nc.vector.tensor_tensor(
                        t[:rows, :, :HEAD_DIM],
                        ps[:rows, 0, :C].rearrange("p (h d) -> p h d", h=H),
                        bv[:rows, :].rearrange("p (h d) -> p h d", h=H),
                        mybir.AluOpType.add)
                    # ones column for the denominator row
                    nc.sync.dma_start(t[:rows, :, HEAD_DIM:HEAD_DIM + 1],
                                      vpd[:rows, :, :])
                    vts.append(t)
                vtss[b] = vts

            pend = []

            def emit_head(b, h):
                qkt, vts, x2t = qkts[b], vtss[b], x2ts[b]
                if True:
                    ti, po = h // 2, 64 * (h % 2)
                    qT = qkt[ti][po:po + 64, :, :].rearrange("p a b -> p (a b)")
                    kT = qkt[3 + ti][po:po + 64, :, :].rearrange("p a b -> p (a b)")
                    # merged mask+bias table for this (batch, head)
                    if ACC_MODE == 'fp8dr':
                        mt = bpool.tile([64, 5, 2, 2, QWP], FP8, tag="mbt",
                                        bufs=3)
                    else:
                        mt = bpool.tile([128, 5, NQ], FP8, tag="mbt", bufs=3)
                    nc.gpsimd.dma_start(mt[:], mbt_d[b, h])
                    ov = o_pool.tile([65, 2, 512], F32, tag="ov")
                    for c, (t0, rows) in enumerate(CHUNKS):
                        kdr = (rows + 1) // 2
                        sp = s_pool.tile([128, 2, 512], F32, tag="sp")
                        mmqk = _mmb if QK_BF16 else _mm
                        for bk in range(2):
                            mmqk(nc, sp[:rows, bk, :QW], kT[:, t0:t0 + rows],
                                 qT[:, QW * bk:QW * (bk + 1)],
                                 start=True, stop=False)
                        for bk in range(2):
                            if ACC_MODE == 'fp8dr':
                                _mmb(nc, sp[:rows, bk, :QW],
                                     idr[:kdr, :, :rows],
                                     mt[:kdr, c, bk, :, :QW],
                                     perf_mode=DR, start=False, stop=True)
                            else:
                                _mmb(nc, sp[:rows, bk, :QW],
                                     ident[:rows, :rows],
                                     mt[:rows, c, QW * bk:QW * (bk + 1)],
                                     start=False, stop=True)
                        # exp
                        pt = ppool.tile([128, 2, QW], F32R, tag="pt")
                        nc.scalar.activation(
                            pt[:rows, :, :], sp[:rows, :, :QW],
                            mybir.ActivationFunctionType.Exp)
                        # P^T @ V'
                        for bk in range(2):
                            _mm(nc, ov[:, bk, :QW], vts[c][:rows, h, :],
                                pt[:rows, bk, :],
                                start=(c == 0), stop=(c == 4))
                    # copy out of PSUM to free the accumulator early
                    ocp = spool.tile([65, 2, QW], F32, tag="ocp", bufs=3)
                    nc.vector.tensor_copy(ocp[:, :, :], ov[:, :, :QW])
                    # defer this head's reciprocal chain by one head so its
                    # scalar-engine ops never sit in the ACT FIFO ahead of
                    # the next head's chunk exps
                    pend.append((x2t, h, ti, po, ocp))
                    if len(pend) > 1:
                        _emit_div(nc, *pend.pop(0))

            def emit_proj(b):
                x2t = x2ts[b]
                tsl = [(0, 128), (128, 128), (256, 128), (384, 128), (512, 65)]
                for i, (t0, tn) in enumerate(tsl):
                    ps = mm_ps(mmctr[0])
                    mmctr[0] += 1
                    for ci in range(3):
                        _mmb(nc, ps[:tn, 0, :C],
                             x2t[ci][:, :, :].rearrange("p a b -> p (a b)")[:, t0:t0 + tn],
                             wproj[ci][:, :], start=(ci == 0), stop=(ci == 2))
                    yt = spool.tile([128, C], F32, tag="yt", bufs=3)
                    nc.vector.tensor_tensor(yt[:tn, :], ps[:tn, 0, :C],
                                            bpj[:tn, :], mybir.AluOpType.add)
                    nc.sync.dma_start(out_d[b, t0:t0 + tn, :], yt[:tn, :])

            # schedule: emit each batch's qkt weight-groups just-in-time
            # before the head pair that consumes them, so attention ramps
            # while the remaining projections stream; hoist batch 1's qkv
            # before batch 0's proj to cover the division-chain tail
            emit_qkv(0)
            emit_qkv(1)     # x^T DMAs for both batches issue up front
            emit_qkt(0, 0)
            emit_qkt(0, 3)
            emit_vts(0)
            emit_head(0, 0)
            emit_qkt(0, 1)
            emit_qkt(0, 4)
            emit_head(0, 1)
            emit_head(0, 2)
            emit_qkt(0, 2)
            emit_qkt(0, 5)
            emit_head(0, 3)
            emit_head(0, 4)
            emit_head(0, 5)
            for oi in (0, 3, 1, 4, 2, 5):
                emit_qkt(1, oi)
            emit_vts(1)
            while pend:
                _emit_div(nc, *pend.pop(0))
            emit_proj(0)
            for h in range(H):
                emit_head(1, h)
            while pend:
                _emit_div(nc, *pend.pop(0))
            emit_proj(1)

    nc.compile()
    return nc


def prep_inputs(x, qkv_w, qkv_b, proj_w, proj_b, rel_pos, rel_pos_index,
                mask, patch_attn):
    x = np.asarray(x, dtype=np.float32)
    qkv_w = np.asarray(qkv_w, dtype=np.float32)
    qkv_b = np.asarray(qkv_b, dtype=np.float32)
    proj_w = np.asarray(proj_w, dtype=np.float32)
    proj_b = np.asarray(proj_b, dtype=np.float32)
    rel_pos = np.asarray(rel_pos, dtype=np.float32)
    mask = np.asarray(mask)

    # x^T padded to 580 query columns (zeros in the pad)
    xT = np.zeros((B, C, NQ), dtype=ml_dtypes.bfloat16)
    xT[:, :, :N] = x.transpose(0, 2, 1)
    W = qkv_w.copy()
    W[:C] *= np.float32(SCALE)
    b2 = qkv_b.copy()
    b2[:C] *= np.float32(SCALE)
    qkv_wT = np.ascontiguousarray(W.T.astype(ml_dtypes.bfloat16))
    proj_wT = np.ascontiguousarray(proj_w.T.astype(ml_dtypes.bfloat16))
    qkvb_qk = np.ascontiguousarray(b2[:2 * C].reshape(2 * C, 1))
    qkvbv_bc = np.ascontiguousarray(np.broadcast_to(b2[2 * C:], (128, C)))
    projb_bc = np.ascontiguousarray(np.broadcast_to(proj_b, (128, C)))
    vpad = np.ones((128, H, 1), dtype=np.float32)

    # fp8 DoubleRow identity
    k_ = np.arange(64)
    idr = np.zeros((64, 2, 128), dtype=FP8NP)
    for r in range(2):
        idr[k_, r, np.minimum(2 * k_ + r, 127)] = (2 * k_ + r < 128)

    # merged mask + rel-pos bias table, key-major
    MASKVAL = 240.0
    relb = np.zeros((H, N, NQ), dtype=np.float32)
    if patch_attn:
        relb[:, NUM_CLS:, NUM_CLS:N] = rel_pos[:, rel_pos_index.T]
    mb = (mask.transpose(0, 2, 1).astype(np.float32) - 1.0) * MASKVAL  # [B,k,q]
    if ACC_MODE == 'fp8dr':
        # DR-packed: mbt[b,h,j,c,bk,r,col] = bias[b,h, key=t0_c+2j+r,
        #                                         q=290*bk+col]
        mbt = np.empty((B, H, 5, 64, 2, 2, QWP), dtype=FP8NP)
        pk = np.empty((H, 5, 128, 2, QWP), dtype=np.float32)
        for b in range(B):
            pk[:] = 0.0
            for c, (t0, rows) in enumerate(CHUNKS):
                blk = relb[:, t0:t0 + rows, :].copy()
                blk[:, :, :N] += mb[b, t0:t0 + rows, :][None]
                for bk in range(2):
                    pk[:, c, :rows, bk, :QW] = blk[:, :, QW * bk:QW * (bk + 1)]
            # (h, c, key=2j+r, bk, col) -> (h, c, j, r, bk, col)
            pk8 = pk.reshape(H, 5, 64, 2, 2, QWP).astype(FP8NP)
            mbt[b] = pk8.transpose(0, 1, 2, 4, 3, 5)  # swap r <-> bk
        mbt = np.ascontiguousarray(mbt.transpose(0, 1, 3, 2, 4, 5, 6))
    else:
        # mbt[b, h, k, c, q] = bias[b, h, key = t0_c + k, q]
        mbt = np.empty((B, H, 5, 128, NQ), dtype=FP8NP)
        pk = np.empty((H, 5, 128, NQ), dtype=np.float32)
        for b in range(B):
            pk[:] = 0.0
            for c, (t0, rows) in enumerate(CHUNKS):
                pk[:, c, :rows, :] = relb[:, t0:t0 + rows, :]
                pk[:, c, :rows, :N] += mb[b, t0:t0 + rows, :][None]
            mbt[b] = pk.astype(FP8NP)
        mbt = np.ascontiguousarray(mbt.transpose(0, 1, 3, 2, 4))

    shared = {
        "qkv_wT": qkv_wT, "proj_wT": proj_wT,
        "qkvb_qk": qkvb_qk, "qkvbv_bc": qkvbv_bc, "projb_bc": projb_bc,
        "vpad": vpad, "id8": np.eye(128, dtype=FP8NP),
    }
    in_maps = []
    for i in range(NCORES):
        m = dict(shared)
        m["xT"] = np.ascontiguousarray(xT[NB * i:NB * (i + 1)])
        m["mbt"] = np.ascontiguousarray(mbt[NB * i:NB * (i + 1)])
        in_maps.append(m)
    return in_maps


_NC_CACHE = {}


def _get_nc(patch_attn: bool):
    key = (bool(patch_attn), QK_BF16, ACC_MODE, REC_FAST)
    if key not in _NC_CACHE:
        _NC_CACHE[key] = build_program(bool(patch_attn))
    return _NC_CACHE[key]


def kernel(**inputs):
    patch_attn = bool(np.asarray(inputs["patch_attn"]))
    nc = _get_nc(patch_attn)
    in_maps = prep_inputs(**inputs)
    res = bass_utils.run_bass_kernel_spmd(nc, in_maps,
                                          core_ids=list(range(NCORES)))
    out = np.concatenate([res.results[i]["out"] for i in range(NCORES)], axis=0)
    return np.ascontiguousarray(out.astype(np.float32))
